# revision 8
# baseline (speedup 1.0000x reference)
"""Trainium2 Bass kernel for causal self-attention (B=4, S=2048, C=2048, H=16).

Sharding over 8 NeuronCores: core = 2*batch + head_group
  - data-parallel over the 4 batches (outer axis)
  - tensor-parallel over heads within a batch: 2 groups x 8 heads
Each core computes qkv projection for its head group, block-causal
flash-style attention for its 8 heads, and a partial output projection
(contraction over its 1024 w_proj rows). The host sums the two partial
outputs per batch, descales, and adds b_proj.

Precision: row-hybrid fp8. Causal attention makes early rows (few keys)
produce outputs ~10x larger than diffuse rows, so the max-normalized
error budget concentrates there. Rows < RB stay bf16 end-to-end; rows
>= RB use fp8e4 with DoubleRow matmuls (2 contraction chunks per
instruction) for the qkv and output projections. Weights are pre-scaled
x64 on the host so fp8 stays in its normal range; descale is folded
into the exp scale / v copy / host-side reduction. Attention internals
(scores, exp, attn@V) are bf16 with f32 PSUM.
"""

import os
from contextlib import ExitStack

import numpy as np
import ml_dtypes

import concourse.bass as bass
import concourse.tile as tile
from concourse import bacc, mybir
from concourse.bass_utils import run_bass_kernel_spmd

BF16 = mybir.dt.bfloat16
F8 = mybir.dt.float8e4
F32 = mybir.dt.float32
ExpF = mybir.ActivationFunctionType.Exp
DR = mybir.MatmulPerfMode.DoubleRow
NPBF16 = ml_dtypes.bfloat16
NPF8 = ml_dtypes.float8_e4m3

B, S, C, H = 4, 2048, 2048, 16
D = 128
N_CORES = 8
NH = 8              # heads per core
NQ = NH * D         # 1024 q (=k=v) columns per core
SQT = 512           # sq tile width
RB = 256            # rows below this stay bf16 (precision island)
JB = RB // 128      # bf16 s-chunks
WS = 64.0           # host-side weight scale (power of 2)
# "dr": fp8 DoubleRow; "fp8": fp8 without perf mode (bf16-rate)
VARIANT = os.environ.get("KVARIANT", "dr")


def _build(compile=True, reps=1):
    CK = C // 128            # contraction chunks
    CP = CK // 2             # DoubleRow pairs
    NST = S // SQT           # s tiles of 512
    NSC = S // 128           # s chunks of 128
    NVT = NQ // 512          # v n-tiles of 512
    ET = C // 512            # proj e tiles
    scale = 1.0 / float(np.sqrt(float(D))) / (WS * WS)

    nc = bacc.Bacc(
        "TRN2",
        target_bir_lowering=False,
        debug=False,
        enable_asserts=False,
        num_devices=N_CORES,
    )
    xT8_d = nc.dram_tensor("xT8", [128, NSC * CK * 128], F8, kind="ExternalInput").ap()
    xTb_d = nc.dram_tensor("xTb", [128, JB * CK * 128], BF16, kind="ExternalInput").ap()
    wqkv8_d = nc.dram_tensor("wqkv8", [C, 3 * NQ], F8, kind="ExternalInput").ap()
    wqkvb_d = nc.dram_tensor("wqkvb", [C, 3 * NQ], BF16, kind="ExternalInput").ap()
    bqkv_d = nc.dram_tensor("bqkv", [1, 3 * NQ], BF16, kind="ExternalInput").ap()
    bqkvcol_d = nc.dram_tensor(
        "bqkvcol", [128, 2 * NQ // 128], BF16, kind="ExternalInput"
    ).ap()
    mtri_d = nc.dram_tensor("mtri", [128, 256], BF16, kind="ExternalInput").ap()
    wproj8_d = nc.dram_tensor("wproj8", [NQ, C], F8, kind="ExternalInput").ap()
    wprojb_d = nc.dram_tensor("wprojb", [NQ, C], BF16, kind="ExternalInput").ap()
    out_d = nc.dram_tensor("out", [S, C], BF16, kind="ExternalOutput").ap()

    with tile.TileContext(nc) as tc, ExitStack() as top:
        top.enter_context(
            nc.allow_low_precision(reason="bf16 rowsum accumulate, sim-validated")
        )
        persist = top.enter_context(tc.tile_pool(name="persist", bufs=1))
        # q_sb/k_sb: [d, h, s] bf16 holding WS*q / WS*k
        q_sb = persist.tile([128, NH, S], BF16, tag="q")
        k_sb = persist.tile([128, NH, S], BF16, tag="k")
        # v_sb: [s%128, s//128, h*128+d] bf16, true scale
        v_sb = persist.tile([128, NSC, NQ], BF16, tag="v")
        mtri_sb = persist.tile([128, 256], BF16, tag="mtri")
        bias_sb = persist.tile([1, 3 * NQ], BF16, tag="bias")
        bias_col = persist.tile([128, 2 * NQ // 128], BF16, tag="bias_col")
        ones_bf = persist.tile([1, 512], BF16, tag="ones_bf")
        ones_col_bf = persist.tile([128, 1], BF16, tag="ones_col_bf")
        ones_row_bf = persist.tile([1, 128], BF16, tag="ones_row_bf")

        nc.sync.dma_start(out=mtri_sb, in_=mtri_d)
        nc.sync.dma_start(out=bias_sb, in_=bqkv_d)
        nc.sync.dma_start(out=bias_col, in_=bqkvcol_d)
        nc.vector.memset(ones_bf, 1.0)
        nc.vector.memset(ones_col_bf, 1.0)
        nc.vector.memset(ones_row_bf, 1.0)
        ltri = mtri_sb[:, 0:128]      # strict lower-tri (sk>sq) 0/1
        negdiag = mtri_sb[:, 128:256]  # diag(-1e9)

        for _rep in range(reps):
            # ---------------- Phase 1: QKV projection ----------------
            with (
                tc.tile_pool(name="ph1x", bufs=1) as ph1x,
                tc.tile_pool(name="ph1wv", bufs=1) as ph1wv,
                tc.tile_pool(name="ph1wqk", bufs=3) as ph1wqk,
                tc.tile_pool(name="ps1", bufs=4, space="PSUM") as ps1,
            ):
                def load_wv(nt):
                    w8 = ph1wv.tile([128, CK, 512], F8, tag="wv8", bufs=2)
                    nc.sync.dma_start(
                        out=w8,
                        in_=wqkv8_d[:, 2 * NQ + nt * 512 : 2 * NQ + (nt + 1) * 512]
                        .rearrange("(ck p) n -> p ck n", p=128),
                    )
                    wb = ph1wv.tile([128, CK, 512], BF16, tag="wvb", bufs=1)
                    nc.scalar.dma_start(
                        out=wb,
                        in_=wqkvb_d[:, 2 * NQ + nt * 512 : 2 * NQ + (nt + 1) * 512]
                        .rearrange("(ck p) n -> p ck n", p=128),
                    )
                    return w8, wb

                wv0 = load_wv(0)  # ahead of the x stream in the DMA queue
                xfull = ph1x.tile([128, CK, S], F8, tag="xf")
                xb_sb = ph1x.tile([128, CK, RB], BF16, tag="xb")
                dma_engs = [nc.sync, nc.scalar]
                for sc in range(JB):
                    nc.scalar.dma_start(
                        out=xb_sb[:, :, bass.ts(sc, 128)],
                        in_=xTb_d[:, sc * CK * 128 : (sc + 1) * CK * 128].rearrange(
                            "p (ck sl) -> p ck sl", ck=CK
                        ),
                    )
                for sc in range(NSC):
                    dma_engs[sc % 2].dma_start(
                        out=xfull[:, :, bass.ts(sc, 128)],
                        in_=xT8_d[:, sc * CK * 128 : (sc + 1) * CK * 128].rearrange(
                            "p (ck sl) -> p ck sl", ck=CK
                        ),
                    )

                def emit_qk(sec, hh):
                    nb = sec * NH + hh
                    w8 = ph1wqk.tile([128, CK, 128], F8, tag="wqk8", name="wqk8")
                    nc.sync.dma_start(
                        out=w8,
                        in_=wqkv8_d[:, bass.ts(nb, 128)].rearrange(
                            "(ck p) n -> p ck n", p=128
                        ),
                    )
                    wb = ph1wqk.tile([128, CK, 128], BF16, tag="wqkb", name="wqkb")
                    nc.scalar.dma_start(
                        out=wb,
                        in_=wqkvb_d[:, bass.ts(nb, 128)].rearrange(
                            "(ck p) n -> p ck n", p=128
                        ),
                    )
                    dest = q_sb if sec == 0 else k_sb
                    for st in range(NST):
                        ps = ps1.tile([128, 512], F32, tag="psqk", bufs=4, name="psqk")
                        def qk_f8(ps_ap, ssl):
                            if VARIANT == "dr":
                                for cp in range(CP):
                                    nc.tensor.matmul(
                                        ps_ap,
                                        lhsT=w8[:, 2 * cp : 2 * cp + 2, :],
                                        rhs=xfull[:, 2 * cp : 2 * cp + 2, ssl],
                                        start=(cp == 0),
                                        stop=(cp == CP - 1),
                                        perf_mode=DR,
                                    )
                            else:
                                for ck in range(CK):
                                    nc.tensor.matmul(
                                        ps_ap,
                                        lhsT=w8[:, ck, :],
                                        rhs=xfull[:, ck, ssl],
                                        start=(ck == 0),
                                        stop=(ck == CK - 1),
                                    )

                        if st == 0:
                            # bf16 island: output cols 0:RB
                            for ck in range(CK):
                                nc.tensor.matmul(
                                    ps[:, 0:RB],
                                    lhsT=wb[:, ck, :],
                                    rhs=xb_sb[:, ck, :],
                                    start=(ck == 0),
                                    stop=(ck == CK - 1),
                                )
                            qk_f8(ps[:, RB:512], slice(RB, 512))
                        else:
                            qk_f8(ps, bass.ts(st, 512))
                        nc.scalar.add(
                            dest[:, hh, bass.ts(st, 512)], ps, bias_col[:, nb : nb + 1]
                        )

                # v: n-tiles of 512, psum[s 128, n 512]
                for nt in range(NVT):
                    w8, wb = wv0 if nt == 0 else load_wv(nt)
                    for sc in range(NSC):
                        ps = ps1.tile([128, 512], F32, tag="psv", bufs=4)
                        if sc < JB:
                            for ck in range(CK):
                                nc.tensor.matmul(
                                    ps,
                                    lhsT=xb_sb[:, ck, bass.ts(sc, 128)],
                                    rhs=wb[:, ck, :],
                                    start=(ck == 0),
                                    stop=False,
                                )
                        elif VARIANT == "dr":
                            for cp in range(CP):
                                nc.tensor.matmul(
                                    ps,
                                    lhsT=xfull[:, 2 * cp : 2 * cp + 2, bass.ts(sc, 128)],
                                    rhs=w8[:, 2 * cp : 2 * cp + 2, :],
                                    start=(cp == 0),
                                    stop=False,
                                    perf_mode=DR,
                                )
                        else:
                            for ck in range(CK):
                                nc.tensor.matmul(
                                    ps,
                                    lhsT=xfull[:, ck, bass.ts(sc, 128)],
                                    rhs=w8[:, ck, :],
                                    start=(ck == 0),
                                    stop=False,
                                )
                        # bias: out[s, n] += 1 * (WS*b[n])
                        nc.tensor.matmul(
                            ps,
                            lhsT=ones_bf[:, :128],
                            rhs=bias_sb[:, 2 * NQ + nt * 512 : 2 * NQ + (nt + 1) * 512],
                            start=False,
                            stop=True,
                        )
                        # descale 1/WS during copy
                        nc.vector.tensor_scalar_mul(
                            v_sb[:, sc, bass.ts(nt, 512)], ps, 1.0 / WS
                        )
                # k then q, transposed: psum[n 128, s 512]
                for hh in range(NH):
                    emit_qk(1, hh)
                for hh in range(NH):
                    emit_qk(0, hh)

            # -------- Phase 2+3: block-causal attention + projection --------
            with (
                tc.tile_pool(name="att", bufs=4) as att,
                tc.tile_pool(name="ph3", bufs=2) as ph3,
                tc.tile_pool(name="ps2", bufs=1, space="PSUM") as ps2,
            ):
                # y storage: bf16 island rows < RB, fp8 rows >= RB
                y_bf = ph3.tile([128, NH, RB], BF16, tag="ybf", bufs=1)
                y8 = ph3.tile([128, NH, S], F8, tag="y8", bufs=1)
                wp8 = ph3.tile([128, NH, C], F8, tag="wp8", bufs=1)
                nc.sync.dma_start(out=wp8, in_=wproj8_d.rearrange("(h p) e -> p h e", p=128))
                wpb = ph3.tile([128, NH, C], BF16, tag="wpb", bufs=1)
                nc.scalar.dma_start(out=wpb, in_=wprojb_d.rearrange("(h p) e -> p h e", p=128))

                def emit_proj(t_src, lo, hi, tag="po", bufs=1):
                    tiles = [
                        (sqc, et)
                        for sqc in range(4 * t_src, 4 * (t_src + 1))
                        for et in range(ET)
                    ]
                    for sqc, et in tiles[lo:hi]:
                        ps_o = ps2.tile([128, 512], F32, tag=tag, bufs=bufs)
                        if sqc < JB:
                            for hp in range(NH):
                                nc.tensor.matmul(
                                    ps_o,
                                    lhsT=y_bf[:, hp, bass.ts(sqc, 128)],
                                    rhs=wpb[:, hp, bass.ts(et, 512)],
                                    start=(hp == 0),
                                    stop=(hp == NH - 1),
                                )
                        elif VARIANT == "dr":
                            for hp in range(NH // 2):
                                nc.tensor.matmul(
                                    ps_o,
                                    lhsT=y8[:, 2 * hp : 2 * hp + 2, bass.ts(sqc, 128)],
                                    rhs=wp8[:, 2 * hp : 2 * hp + 2, bass.ts(et, 512)],
                                    start=(hp == 0),
                                    stop=(hp == NH // 2 - 1),
                                    perf_mode=DR,
                                )
                        else:
                            for hp in range(NH):
                                nc.tensor.matmul(
                                    ps_o,
                                    lhsT=y8[:, hp, bass.ts(sqc, 128)],
                                    rhs=wp8[:, hp, bass.ts(et, 512)],
                                    start=(hp == 0),
                                    stop=(hp == NH - 1),
                                )
                        o_sb = ph3.tile([128, 512], BF16, tag="o")
                        nc.vector.tensor_copy(o_sb, ps_o)
                        nc.sync.dma_start(
                            out=out_d[bass.ts(sqc, 128), bass.ts(et, 512)], in_=o_sb
                        )

                for t in range(NST):
                    tsl = bass.ts(t, SQT)
                    nsk = 4 * t + 4  # block-causal sk chunks
                    pending = None   # previous head awaiting normalization

                    def flush_pending(t=t, tsl=tsl):
                        nonlocal pending
                        if pending is None:
                            return
                        yu_p, rs_p, h_p = pending
                        ps_bc = ps2.tile([128, 512], F32, tag="bc", bufs=1)
                        nc.tensor.matmul(
                            ps_bc, lhsT=ones_row_bf, rhs=rs_p, start=True, stop=True
                        )
                        bc_sb = att.tile([128, 512], F32, tag="bcs", bufs=2)
                        nc.vector.tensor_copy(bc_sb, ps_bc)
                        if t == 0:
                            nc.vector.tensor_mul(
                                y_bf[:, h_p, 0:RB], yu_p[:, 0:RB], bc_sb[:, 0:RB]
                            )
                            nc.vector.tensor_mul(
                                y8[:, h_p, RB:512], yu_p[:, RB:512], bc_sb[:, RB:512]
                            )
                        else:
                            nc.vector.tensor_mul(y8[:, h_p, tsl], yu_p, bc_sb)
                        pending = None

                    for h in range(NH):
                        ps_yu = ps2.tile([128, 512], F32, tag="yu", bufs=2)
                        ps_rs = ps2.tile([1, 512], F32, tag="rs", bufs=1)
                        # row-sum partials on DVE; two interleaved chains for long
                        # blocks so the serial adds stay shorter than PE's work
                        acc = att.tile([128, 512], BF16, tag="acc", bufs=2, name="acc")
                        acc2 = None
                        if nsk > 8:
                            acc2 = att.tile([128, 512], BF16, tag="acc2", bufs=2, name="acc2")
                        sc_tiles = {}

                        def emit_scores(j, h=h, t=t):
                            off = 0 if j < 4 * t else (j - 4 * t) * 128
                            w = 512 - off
                            diag = j >= 4 * t
                            ps_sc = ps2.tile([128, 512], F32, tag="sc", bufs=3)
                            # scoresT[sk, sq] = k_h.T q_h (live sq columns only)
                            nc.tensor.matmul(
                                ps_sc[:, :w],
                                lhsT=k_sb[:, h, bass.ts(j, 128)],
                                rhs=q_sb[:, h, t * SQT + off : (t + 1) * SQT],
                                start=True,
                                stop=not diag,
                            )
                            if diag:
                                # causal mask on PE: scoresT[p, f] += -1e9 * (p > f)
                                nc.tensor.matmul(
                                    ps_sc[:, :128],
                                    lhsT=negdiag,
                                    rhs=ltri,
                                    start=False,
                                    stop=True,
                                )
                            sc_tiles[j] = (ps_sc, off, w)

                        emit_scores(0)
                        if nsk > 1:
                            emit_scores(1)
                        for j in range(nsk):
                            ps_sc, off, w = sc_tiles.pop(j)
                            e = att.tile([128, 512], BF16, tag="e", bufs=10)
                            nc.scalar.activation(
                                out=e[:, off:], in_=ps_sc[:, :w], func=ExpF, scale=scale
                            )
                            if j + 2 < nsk:
                                emit_scores(j + 2)
                            if j == 0:
                                flush_pending()
                            # row sums (live region; first touch is full width)
                            tgt = acc if (acc2 is None or j % 2 == 0) else acc2
                            if j <= (0 if acc2 is None else 1):
                                nc.vector.tensor_copy(tgt, e)
                            else:
                                nc.vector.tensor_add(
                                    tgt[:, off:], tgt[:, off:], e[:, off:]
                                )
                            # yu[d, sq] += v[sk, d].T @ e[sk, sq] (live region)
                            nc.tensor.matmul(
                                ps_yu[:, off:],
                                lhsT=v_sb[:, j, bass.ts(h, 128)],
                                rhs=e[:, off:],
                                start=(j == 0),
                                stop=(j == nsk - 1),
                            )
                        # partition-reduce the accumulated exp sums on PE
                        nc.tensor.matmul(
                            ps_rs,
                            lhsT=ones_col_bf,
                            rhs=acc,
                            start=True,
                            stop=(acc2 is None),
                        )
                        if acc2 is not None:
                            nc.tensor.matmul(
                                ps_rs, lhsT=ones_col_bf, rhs=acc2, start=False, stop=True
                            )
                        rs_sb = att.tile([1, 512], BF16, tag="rsb", bufs=2)
                        nc.vector.reciprocal(rs_sb, ps_rs)
                        # interleave prev t-block's projection tiles: fills PE
                        # while this head's reciprocal completes on DVE
                        if t > 0:
                            emit_proj(t - 1, 2 * h, 2 * h + 2)
                        pending = (ps_yu, rs_sb, h)
                    flush_pending()
                    if t == NST - 1:
                        emit_proj(t, 0, 4 * ET, tag="yu", bufs=2)

    if compile:
        nc.compile()
    return nc


def _make_mtri():
    """[:, :128] strict lower-tri (sk>sq -> 1); [:, 128:] diag(-1e9)."""
    m = np.zeros((128, 256), np.float32)
    m[:, :128] = np.tril(np.ones((128, 128), np.float32), -1)
    m[:, 128:] = np.diag(np.full(128, -1e9, np.float32))
    return m.astype(NPBF16)


_NC_CACHE = None


def _get_nc():
    global _NC_CACHE
    if _NC_CACHE is None:
        _NC_CACHE = _build()
    return _NC_CACHE


def _swizzle(xb):
    """x [S, C] -> [128, (S//128)*(C//128)*128]: [p, sc, ck, sl]."""
    nsc, nck = xb.shape[0] // 128, xb.shape[1] // 128
    return np.ascontiguousarray(
        xb.reshape(nsc, 128, nck, 128).transpose(3, 0, 2, 1)
    ).reshape(128, nsc * nck * 128)


def _make_in_maps(x, w_qkv, b_qkv, w_proj):
    mtri = _make_mtri()
    in_maps = []
    for core in range(N_CORES):
        b = core // 2
        g = core % 2
        cs = slice(g * NQ, (g + 1) * NQ)
        xb = np.asarray(x[b], np.float32)
        xT8 = _swizzle(xb.astype(NPF8))
        xTb = _swizzle(xb[:RB].astype(NPBF16))
        wqkv_c = np.ascontiguousarray(
            np.concatenate(
                [w_qkv[:, cs], w_qkv[:, C:][:, cs], w_qkv[:, 2 * C:][:, cs]], axis=1
            )
        ) * WS
        bqkv_c = (
            np.concatenate([b_qkv[cs], b_qkv[C:][cs], b_qkv[2 * C:][cs]])[None, :] * WS
        ).astype(NPBF16)
        bqkvcol = np.ascontiguousarray(
            bqkv_c[0, : 2 * NQ].reshape(2 * NQ // 128, 128).T
        )
        wp = np.ascontiguousarray(w_proj[cs, :]) * WS
        in_maps.append(
            {
                "xT8": xT8,
                "xTb": xTb,
                "wqkv8": wqkv_c.astype(NPF8),
                "wqkvb": wqkv_c.astype(NPBF16),
                "bqkv": bqkv_c,
                "bqkvcol": bqkvcol,
                "mtri": mtri,
                "wproj8": wp.astype(NPF8),
                "wprojb": wp.astype(NPBF16),
            }
        )
    return in_maps


def kernel(x, w_qkv, b_qkv, w_proj, b_proj):
    x = np.asarray(x, np.float32)
    w_qkv = np.asarray(w_qkv, np.float32)
    b_qkv = np.asarray(b_qkv, np.float32)
    w_proj = np.asarray(w_proj, np.float32)
    b_proj = np.asarray(b_proj, np.float32)

    nc = _get_nc()
    in_maps = _make_in_maps(x, w_qkv, b_qkv, w_proj)
    res = run_bass_kernel_spmd(nc, in_maps, core_ids=list(range(N_CORES)))

    out = np.empty((B, S, C), np.float32)
    for b in range(B):
        out[b] = (
            res.results[2 * b]["out"].astype(np.float32)
            + res.results[2 * b + 1]["out"].astype(np.float32)
        ) / WS
        out[b] += b_proj[None, :]
    return out
